# revision 38
# baseline (speedup 1.0000x reference)
"""Multi-head attention Trainium2 kernel (8 NeuronCores).

Sharding: data-parallel over batch (4 pairs of cores) x tensor-parallel over
heads (2-way split within each pair). Core c handles batch c//2 and heads
(c%2)*8 .. (c%2)*8+8.

Output projection: instead of row-parallel Wo + ReduceScatter (which leaves a
long serialized tail), each core AllGathers the normalized per-head context
(bf16) within its pair and computes a column shard of y = relu(ctx @ Wo + bo)
with the full D contraction — no post-matmul reduction, and the collective for
q-chunk j overlaps attention on chunk j+1. Core (b, hh) emits y[:, hh*512:
(hh+1)*512] for batch b; the host only concatenates.

Schedule: attention runs over ascending q-chunks j (512 rows each); the
projections for chunk j+1 (k/q/v segment j+1) are interleaved into the
attention kb-loop of chunk j as PE filler work, so the scalar engine (exp)
starts ~20us into the kernel instead of after all projections.

Math notes vs. the reference:
 - reference subtracts the row max (over ALL keys, pre-mask) inside exp and
   adds EPS=1e-7 to the softmax denominator. Since scores = q.k/8 >= 0
   (q,k are post-relu) and bounded (~<6), exp never overflows without the
   max subtraction, every row's denominator is >= 1, and both the max
   subtraction and EPS cancel to < 1e-5 relative. So we compute
   a = exp(s/8)*causal / sum(exp(s/8)*causal) directly.
"""

import numpy as np
import ml_dtypes

B, S, D, H = 4, 2048, 1024, 16
HD = 64          # head dim
HC = 8           # heads per core
DC = HC * HD     # 512 head-dims per core
YC = 512         # output columns per core
NCORES = 8

_cache = {}


def _build(debug_dump=False):
    import concourse.bass as bass
    import concourse.mybir as mybir
    import concourse.tile as tile
    from concourse import bacc
    from concourse.masks import make_upper_triangular

    f32 = mybir.dt.float32
    bf16 = mybir.dt.bfloat16
    AF = mybir.ActivationFunctionType

    nc = bacc.Bacc("TRN2", target_bir_lowering=False, debug=False,
                   num_devices=NCORES)

    xT_d = nc.dram_tensor("xT", [D, S], bf16, kind="ExternalInput")
    wq_d = nc.dram_tensor("wq", [D, DC], bf16, kind="ExternalInput")
    wk_d = nc.dram_tensor("wk", [D, DC], bf16, kind="ExternalInput")
    wv_d = nc.dram_tensor("wv", [D, DC], bf16, kind="ExternalInput")
    wo_d = nc.dram_tensor("wo", [D, YC], bf16, kind="ExternalInput")
    bq_d = nc.dram_tensor("bq", [128, 4], f32, kind="ExternalInput")
    bk_d = nc.dram_tensor("bk", [128, 4], f32, kind="ExternalInput")
    bvb_d = nc.dram_tensor("bvb", [128, DC], f32, kind="ExternalInput")
    bob_d = nc.dram_tensor("bob", [128, YC], f32, kind="ExternalInput")
    y_d = nc.dram_tensor("y", [S, YC], f32, kind="ExternalOutput")
    if debug_dump:
        dagi_d = nc.dram_tensor("dagi", [DC, 512], bf16,
                                kind="ExternalOutput")
        dago_d = nc.dram_tensor("dago", [2 * DC, 512], bf16,
                                kind="ExternalOutput")

    NQT = S // 512          # 4 q-chunks of 512
    NCH = D // 128          # 8 contraction chunks for projections

    lowp = nc.allow_low_precision("bf16 matmul inputs")
    lowp.__enter__()
    with tile.TileContext(nc) as tc:
        with (
            tc.tile_pool(name="const", bufs=1) as cp,
            tc.tile_pool(name="xt", bufs=1) as xp,
            tc.tile_pool(name="proj", bufs=1) as pp,
            tc.tile_pool(name="agt", bufs=2) as agp,
            tc.tile_pool(name="ework", bufs=4) as ep,
            tc.tile_pool(name="small", bufs=2) as sp,
            tc.tile_pool(name="evac", bufs=3) as vp,
            tc.tile_pool(name="ps", bufs=2, space="PSUM") as psp,
            tc.tile_pool(name="ctxps", bufs=2, space="PSUM") as cxp,
            tc.tile_pool(name="opps", bufs=2, space="PSUM") as opp,
            tc.tile_pool(name="dram", bufs=1, space="DRAM") as dp,
        ):
            # ---- constants ----
            tri = cp.tile([128, 128], bf16, name="tri", tag="tri")
            make_upper_triangular(nc, tri[:], val=1.0, diag=True)
            ones_f = cp.tile([128, 64], bf16, name="ones_f", tag="ones_f")
            nc.vector.memset(ones_f[:], 1.0)

            bq_t = cp.tile([128, 4], f32, name="bq", tag="bq")
            nc.gpsimd.dma_start(bq_t[:], bq_d[:])
            bk_t = cp.tile([128, 4], f32, name="bk", tag="bk")
            nc.gpsimd.dma_start(bk_t[:], bk_d[:])
            bvb_t = cp.tile([128, DC], f32, name="bvb", tag="bvb")
            bob_t = cp.tile([128, YC], f32, name="bob", tag="bob")

            # ---- weight + x loads ----
            # One 512KB DMA per x chunk (fewer issues = no DMA-ring
            # stalls); sync interleaves wk[c]/x[c] so the first kT group
            # starts as soon as chunk 0 lands. Nothing is issued on the
            # scalar queue (DMA issues there would block the first exp).
            # x chunks as two half-tiles [128, 1024] on sync (the first
            # halves unblock seg-0/1 projections after ~2MB); all weights
            # stream on gpsimd in parallel.
            xth = [[None, None] for _ in range(NCH)]
            for half in range(2):
                for c in range(NCH):
                    t = xp.tile([128, 1024], bf16, name=f"xt{c}_{half}",
                                tag=f"xt{c}_{half}")
                    nc.sync.dma_start(
                        t[:], xT_d[c * 128:(c + 1) * 128,
                                   half * 1024:(half + 1) * 1024])
                    xth[c][half] = t

            def xseg(c, seg):
                return xth[c][seg // 2][:, (seg % 2) * 512:
                                        (seg % 2) * 512 + 512]

            def xst(c, st):
                return xth[c][st // 8][:, (st % 8) * 128:(st % 8) * 128 + 128]

            wkch, wqch, wvch = [], [], []
            for lst, nm, w_d in ((wkch, "k", wk_d), (wqch, "q", wq_d),
                                 (wvch, "v", wv_d)):
                for c in range(NCH):
                    lst.append(pp.tile([128, DC], bf16, name=f"w{nm}{c}",
                                       tag=f"w{nm}{c}"))
            for lst, w_d in ((wkch, wk_d), (wqch, wq_d)):
                for c in range(NCH):
                    nc.gpsimd.dma_start(lst[c][:],
                                        w_d[c * 128:(c + 1) * 128, :])
            nc.gpsimd.dma_start(bvb_t[:], bvb_d[:])
            nc.gpsimd.dma_start(bob_t[:], bob_d[:])
            for c in range(NCH):
                nc.gpsimd.dma_start(wvch[c][:], wv_d[c * 128:(c + 1) * 128, :])
            woch = []
            for g in range(NCH):
                wt = pp.tile([128, YC], bf16, name=f"wo{g}", tag=f"wo{g}")
                nc.gpsimd.dma_start(wt[:], wo_d[g * 128:(g + 1) * 128, :])
                woch.append(wt)
            onesr = cp.tile([1, 128], bf16, name="onesr", tag="onesr")
            nc.vector.memset(onesr[:], 1.0)
            bobr = cp.tile([1, YC], bf16, name="bobr", tag="bobr")
            nc.vector.tensor_copy(bobr[:], bob_t[0:1, :])

            # ---- persistent projection outputs ----
            # kT/qT tile t holds local heads 2t (partitions 0:64) and
            # 2t+1 (64:128), full sequence.
            kT = [pp.tile([128, S], bf16, name=f"kT{t}", tag=f"kT{t}")
                  for t in range(4)]
            qT = [pp.tile([128, S], bf16, name=f"qT{t}", tag=f"qT{t}")
                  for t in range(4)]
            # v in augmented layout [128, HC, 65]: columns h*65..h*65+63 are
            # relu(x@wv+bv) for local head h; column h*65+64 is 1.0 (gives the
            # softmax row sums for free in the AV matmul).
            vav = [pp.tile([128, HC, 65], bf16, name=f"va{st}", tag=f"va{st}")
                   for st in range(S // 128)]

            def kq_group(wch, bias_t, out, t, seg):
                ps = psp.tile([128, 1024], f32, name="ps", tag="sc", bufs=2)
                for c in range(NCH):
                    nc.tensor.matmul(
                        ps[:, 0:512],
                        wch[c][:, t * 128:(t + 1) * 128],
                        xseg(c, seg),
                        start=(c == 0), stop=(c == NCH - 1),
                    )
                nc.vector.tensor_scalar(
                    out[:, seg * 512:(seg + 1) * 512], ps[:, 0:512],
                    bias_t[:, t:t + 1], 0.0,
                    mybir.AluOpType.add, mybir.AluOpType.max,
                )

            def v_group(st):
                ps = psp.tile([128, 1024], f32, name="ps", tag="sc", bufs=2)
                for c in range(NCH):
                    nc.tensor.matmul(
                        ps[:, 0:512],
                        xst(c, st),
                        wvch[c][:],
                        start=(c == 0), stop=(c == NCH - 1),
                    )
                nc.vector.tensor_add(ps[:, 0:512], ps[:, 0:512], bvb_t[:])
                nc.vector.tensor_scalar(
                    vav[st][:, :, 0:64],
                    ps[:, 0:512].rearrange("p (h d) -> p h d", h=HC),
                    0.0, None, mybir.AluOpType.max,
                )
                nc.vector.tensor_copy(
                    vav[st][:, :, 64:65],
                    ones_f[:, 0:8].rearrange("p (h o) -> p h o", o=1))

            # ---- AllGather plumbing for the output projection ----
            # Chunks 0-2 use one full AllGather each, triggered one j-block
            # late (the input is complete by then and the serialized
            # collective chain stays short). Chunk 3 uses per-p
            # pair-AllGathers fired as each p-block's normalize lands, so
            # the tail only waits for the last small one. Gathered rows are
            # global dc order (even core first) — rank-uniform.
            ag_in = [dp.tile([DC, 512], bf16, name=f"agi{j}", tag=f"agi{j}")
                     for j in range(NQT)]
            ag_full = [dp.tile([2 * DC, 512], bf16, name=f"agf{j}",
                               tag=f"agf{j}") for j in range(3)]
            ag_out3 = [dp.tile([256, 512], bf16, name=f"ago3_{p}",
                               tag=f"ago3_{p}") for p in range(4)]
            agt = {}
            RG = [[0, 1], [2, 3], [4, 5], [6, 7]]

            def ag_chunk_full(j):
                nc.gpsimd.collective_compute(
                    "AllGather", bass.mybir.AluOpType.bypass,
                    replica_groups=RG,
                    ins=[ag_in[j][:].opt()],
                    outs=[ag_full[j][:].opt()],
                )
                for g in range(NCH):
                    t = agp.tile([128, 512], bf16, name=f"agt{j}_{g}",
                                 tag=f"agt{g}", bufs=3)
                    nc.sync.dma_start(t[:], ag_full[j][g * 128:(g + 1) * 128, :])
                    agt[(j, g)] = t

            def ag_chunk3(p):
                nc.gpsimd.collective_compute(
                    "AllGather", bass.mybir.AluOpType.bypass,
                    replica_groups=RG,
                    ins=[ag_in[3][p * 128:(p + 1) * 128, :].opt()],
                    outs=[ag_out3[p][:].opt()],
                )
                for (g, lo) in ((p, 0), (4 + p, 128)):
                    t = agp.tile([128, 512], bf16, name=f"agt3_{g}",
                                 tag=f"agt{g}", bufs=3)
                    nc.sync.dma_start(t[:], ag_out3[p][lo:lo + 128, :])
                    agt[(3, g)] = t

            def attention(j, p, fillers):
                nblk = 4 * j + 4
                ctxA = cxp.tile([65, 512], f32, name="ctx", tag="ctx", bufs=2)
                ctxB = cxp.tile([65, 512], f32, name="ctx", tag="ctx", bufs=2)
                for kb in range(nblk):
                    dlt = kb * 128 - j * 512
                    qoff = max(dlt, 0)
                    w = 512 - qoff
                    qlo = j * 512 + qoff
                    sc = psp.tile([128, 1024], f32, name="sc",
                                  tag="sc", bufs=2)
                    e = ep.tile([128, 1024], bf16, name="e", tag="e", bufs=8)
                    for (hh, tpos) in ((0, (0, 0)), (1, (64, 0))):
                        plo = hh * 64
                        nc.tensor.matmul(
                            sc[:, hh * 512:hh * 512 + w],
                            kT[p][plo:plo + 64, kb * 128:(kb + 1) * 128],
                            qT[p][plo:plo + 64, qlo:qlo + w],
                            start=True, stop=True, tile_position=tpos,
                        )
                    nc.scalar.activation(
                        e[:].rearrange("p (h q) -> p h q", h=2)[:, :, 0:w],
                        sc[:].rearrange("p (h q) -> p h q", h=2)[:, :, 0:w],
                        AF.Exp, bias=0.0, scale=0.125,
                    )
                    if dlt >= 0:
                        nc.vector.tensor_mul(e[:, 0:128], e[:, 0:128], tri[:])
                        nc.vector.tensor_mul(
                            e[:, 512:640], e[:, 512:640], tri[:])
                    if fillers:
                        fillers.pop(0)()
                    for (hh, ctx) in ((0, ctxA), (1, ctxB)):
                        nc.tensor.matmul(
                            ctx[:, qoff:qoff + w],
                            vav[kb][:, 2 * p + hh, :],
                            e[:, hh * 512:hh * 512 + w],
                            start=(kb == 0), stop=(kb == nblk - 1),
                            skip_group_check=True,
                        )
                # normalize: evacuate psum fast with one copy, then divide by
                # the row sums (partition 64) and emit bf16 ctx for the
                # AllGather.
                # normalize. The partition-broadcast of 1/rowsum runs as a
                # rank-1 f32r matmul on the PE (the gpsimd queue blocks on
                # collective ordering and must not carry normalize work).
                # evacuate each ctx psum bank with one copy (frees the bank
                # for the next p-block's AV immediately), then divide by the
                # row sums (partition 64).
                cx = vp.tile([128, 512], bf16, name="cx", tag="cx", bufs=3)
                for (hh, ctx) in ((0, ctxA), (1, ctxB)):
                    cu = sp.tile([64, 512], f32, name="cu", tag="cu", bufs=4)
                    nc.vector.tensor_copy(cu[:], ctx[0:64, :])
                    rho = sp.tile([1, 512], f32, name="rho", tag="rho",
                                  bufs=4)
                    nc.vector.tensor_copy(rho[:], ctx[64:65, :])
                    rc1 = sp.tile([1, 512], f32, name="rc1", tag="rc1",
                                  bufs=4)
                    nc.vector.reciprocal_approx_fast(rc1[:], rho[:])
                    rcp = sp.tile([64, 512], f32, name="rcp", tag="rcp",
                                  bufs=4)
                    nc.gpsimd.partition_broadcast(rcp[:], rc1[:])
                    nc.vector.tensor_mul(
                        cx[hh * 64:hh * 64 + 64, :], cu[:], rcp[:])
                nc.sync.dma_start(ag_in[j][p * 128:(p + 1) * 128, :], cx[:])

            def o_proj_group(jj, qt, tail=False):
                # contract the p=3 chunks (g=3,7) last: their AllGather is
                # the most recent and may still be in flight
                op = opp.tile([128, YC], f32, name="op", tag="op", bufs=2)
                gorder = (0, 1, 2, 4, 5, 6, 3, 7)
                for (i, g) in enumerate(gorder):
                    nc.tensor.matmul(
                        op[:],
                        agt[(jj, g)][:, qt * 128:(qt + 1) * 128],
                        woch[g][:],
                        start=(i == 0), stop=False,
                    )
                # bias via rank-1 matmul; frees the vector engine of the add
                nc.tensor.matmul(op[:], onesr[:], bobr[:],
                                 start=False, stop=True)
                ys = vp.tile([128, YC], f32, name="ys", tag="ys", bufs=2)
                if tail:
                    # scalar is idle after the last exp; vector is not
                    nc.scalar.activation(ys[:], op[:], AF.Relu,
                                         bias=0.0, scale=1.0)
                else:
                    nc.vector.tensor_scalar(
                        ys[:], op[:], 0.0, None, mybir.AluOpType.max)
                nc.sync.dma_start(
                    y_d[jj * 512 + qt * 128:jj * 512 + (qt + 1) * 128, :],
                    ys[:])

            # ---- main pipeline ----
            def K(t, seg):
                return lambda: kq_group(wkch, bk_t, kT[t], t, seg)

            def Q(t, seg):
                return lambda: kq_group(wqch, bq_t, qT[t], t, seg)

            def V(st):
                return lambda: v_group(st)

            def both(a, b):
                def f():
                    a()
                    b()
                return f

            # attention(j,p) needs kT[p]/qT[p] segs 0..j and vav 0..4j+3;
            # scores(j,p) only touch tile p, so attention(0,0) can start
            # after just K(0,0)+Q(0,0). V(kb) fillers are emitted before the
            # AV matmul of the same kb.
            K(0, 0)()
            Q(0, 0)()
            fill_plan = {
                (0, 0): [both(V(0), K(1, 0)), both(V(1), Q(1, 0)),
                         both(V(2), K(2, 0)), both(V(3), Q(2, 0))],
                (0, 1): [K(3, 0), Q(3, 0), V(4), V(5)],
                (0, 2): [K(0, 1), K(1, 1), Q(0, 1), Q(1, 1)],
                (0, 3): [K(2, 1), K(3, 1), Q(2, 1), Q(3, 1)],
                (1, 0): [V(6), V(7), K(0, 2)],
                (1, 1): [K(1, 2), K(2, 2), K(3, 2)],
                (1, 2): [Q(0, 2), Q(1, 2), Q(2, 2)],
                (1, 3): [Q(3, 2), V(8), V(9)],
                (2, 0): [V(10), V(11), K(0, 3)],
                (2, 1): [K(1, 3), K(2, 3), K(3, 3)],
                (2, 2): [Q(0, 3), Q(1, 3), Q(2, 3)],
                (2, 3): [Q(3, 3), V(12), V(13)],
                (3, 0): [V(14), V(15)],
            }

            # Collective triggers for chunk j fire at the START of block
            # (j+1, p): the serialized collective chain (each trigger waits
            # for the previous collective AND the first one waits for the
            # cross-core startup barrier, ~80us) then never blocks the
            # gpsimd queue ahead of normalize broadcasts. j=3's fire
            # immediately (the chain is warm and spaced by then).
            # o-proj placement: chunks 0-2 are deferred into the LATER,
            # scalar-bound attention phases (j=2 gets chunk 0, j=3 gets
            # chunks 1+2) where the PE has idle slots under the exp stream.
            for j in range(NQT):
                for p in range(4):
                    if j > 0 and p == 0:
                        ag_chunk_full(j - 1)
                    attention(j, p, fill_plan.get((j, p), []))
                    if j == 3:
                        ag_chunk3(p)
                        o_proj_group(1, p)
                        o_proj_group(2, p)
                    elif j == 2:
                        o_proj_group(0, p)
            for qt in range(4):
                o_proj_group(3, qt, tail=True)
            if debug_dump:
                nc.scalar.dma_start(dagi_d[:], ag_in[0][:])
                nc.scalar.dma_start(dago_d[:], ag_out[0][:])

    lowp.__exit__(None, None, None)
    nc.compile()
    return nc


def _get_nc():
    if "nc" not in _cache:
        _cache["nc"] = _build()
    return _cache["nc"]


def kernel(x, Wq, bq, Wk, bk, Wv, bv, Wo, bo, trace=False):
    from concourse.bass_utils import run_bass_kernel_spmd

    x = np.asarray(x, np.float32)
    Wq, bq = np.asarray(Wq, np.float32), np.asarray(bq, np.float32)
    Wk, bk = np.asarray(Wk, np.float32), np.asarray(bk, np.float32)
    Wv, bv = np.asarray(Wv, np.float32), np.asarray(bv, np.float32)
    Wo, bo = np.asarray(Wo, np.float32), np.asarray(bo, np.float32)

    nc = _get_nc()
    in_maps = []
    for c in range(NCORES):
        b, hh = c // 2, c % 2
        sl = slice(hh * DC, (hh + 1) * DC)
        yc = slice(hh * YC, (hh + 1) * YC)
        in_maps.append({
            "xT": np.ascontiguousarray(x[b].T).astype(ml_dtypes.bfloat16),
            "wq": np.ascontiguousarray(Wq[:, sl]).astype(ml_dtypes.bfloat16),
            "wk": np.ascontiguousarray(Wk[:, sl]).astype(ml_dtypes.bfloat16),
            "wv": np.ascontiguousarray(Wv[:, sl]).astype(ml_dtypes.bfloat16),
            "wo": np.ascontiguousarray(Wo[:, yc]).astype(ml_dtypes.bfloat16),
            "bq": np.ascontiguousarray(bq[sl].reshape(4, 128).T),
            "bk": np.ascontiguousarray(bk[sl].reshape(4, 128).T),
            "bvb": np.ascontiguousarray(
                np.broadcast_to(bv[sl], (128, DC))),
            "bob": np.ascontiguousarray(
                np.broadcast_to(bo[yc], (128, YC))),
        })

    res = run_bass_kernel_spmd(nc, in_maps, core_ids=list(range(NCORES)),
                               trace=trace)
    _cache["last_result"] = res

    y = np.empty((B, S, D), np.float32)
    for c in range(NCORES):
        b, hh = c // 2, c % 2
        y[b, :, hh * YC:(hh + 1) * YC] = res.results[c]["y"]
    return y


# revision 41
# speedup vs baseline: 1.0468x; 1.0468x over previous
"""Multi-head attention Trainium2 kernel (8 NeuronCores).

Sharding: data-parallel over batch (4 pairs of cores) x tensor-parallel over
heads (2-way split within each pair). Core c handles batch c//2 and heads
(c%2)*8 .. (c%2)*8+8.

Output projection: instead of row-parallel Wo + ReduceScatter (which leaves a
long serialized tail), each core AllGathers the normalized per-head context
(bf16) within its pair and computes a column shard of y = relu(ctx @ Wo + bo)
with the full D contraction — no post-matmul reduction, and the collective for
q-chunk j overlaps attention on chunk j+1. Core (b, hh) emits y[:, hh*512:
(hh+1)*512] for batch b; the host only concatenates.

Schedule: attention runs over ascending q-chunks j (512 rows each); the
projections for chunk j+1 (k/q/v segment j+1) are interleaved into the
attention kb-loop of chunk j as PE filler work, so the scalar engine (exp)
starts ~20us into the kernel instead of after all projections.

Math notes vs. the reference:
 - reference subtracts the row max (over ALL keys, pre-mask) inside exp and
   adds EPS=1e-7 to the softmax denominator. Since scores = q.k/8 >= 0
   (q,k are post-relu) and bounded (~<6), exp never overflows without the
   max subtraction, every row's denominator is >= 1, and both the max
   subtraction and EPS cancel to < 1e-5 relative. So we compute
   a = exp(s/8)*causal / sum(exp(s/8)*causal) directly.
"""

import numpy as np
import ml_dtypes

B, S, D, H = 4, 2048, 1024, 16
HD = 64          # head dim
HC = 8           # heads per core
DC = HC * HD     # 512 head-dims per core
YC = 512         # output columns per core
NCORES = 8

_cache = {}


def _build(debug_dump=False):
    import concourse.bass as bass
    import concourse.mybir as mybir
    import concourse.tile as tile
    from concourse import bacc
    from concourse.masks import make_upper_triangular

    f32 = mybir.dt.float32
    bf16 = mybir.dt.bfloat16
    AF = mybir.ActivationFunctionType

    nc = bacc.Bacc("TRN2", target_bir_lowering=False, debug=False,
                   num_devices=NCORES)

    xT_d = nc.dram_tensor("xT", [D, S], bf16, kind="ExternalInput")
    wq_d = nc.dram_tensor("wq", [D, DC], bf16, kind="ExternalInput")
    wk_d = nc.dram_tensor("wk", [D, DC], bf16, kind="ExternalInput")
    wv_d = nc.dram_tensor("wv", [D, DC], bf16, kind="ExternalInput")
    wo_d = nc.dram_tensor("wo", [D, YC], bf16, kind="ExternalInput")
    bq_d = nc.dram_tensor("bq", [128, 4], f32, kind="ExternalInput")
    bk_d = nc.dram_tensor("bk", [128, 4], f32, kind="ExternalInput")
    bvb_d = nc.dram_tensor("bvb", [128, DC], f32, kind="ExternalInput")
    bob_d = nc.dram_tensor("bob", [128, YC], f32, kind="ExternalInput")
    y_d = nc.dram_tensor("y", [S, YC], f32, kind="ExternalOutput")
    if debug_dump:
        dagi_d = nc.dram_tensor("dagi", [DC, 512], bf16,
                                kind="ExternalOutput")
        dago_d = nc.dram_tensor("dago", [2 * DC, 512], bf16,
                                kind="ExternalOutput")

    NQT = S // 512          # 4 q-chunks of 512
    NCH = D // 128          # 8 contraction chunks for projections

    lowp = nc.allow_low_precision("bf16 matmul inputs")
    lowp.__enter__()
    with tile.TileContext(nc) as tc:
        with (
            tc.tile_pool(name="const", bufs=1) as cp,
            tc.tile_pool(name="xt", bufs=1) as xp,
            tc.tile_pool(name="proj", bufs=1) as pp,
            tc.tile_pool(name="agt", bufs=2) as agp,
            tc.tile_pool(name="ework", bufs=4) as ep,
            tc.tile_pool(name="small", bufs=2) as sp,
            tc.tile_pool(name="evac", bufs=3) as vp,
            tc.tile_pool(name="ps", bufs=2, space="PSUM") as psp,
            tc.tile_pool(name="ctxps", bufs=2, space="PSUM") as cxp,
            tc.tile_pool(name="opps", bufs=2, space="PSUM") as opp,
            tc.tile_pool(name="dram", bufs=1, space="DRAM") as dp,
        ):
            # ---- constants ----
            tri = cp.tile([128, 128], bf16, name="tri", tag="tri")
            make_upper_triangular(nc, tri[:], val=1.0, diag=True)
            ones_f = cp.tile([128, 64], bf16, name="ones_f", tag="ones_f")
            nc.vector.memset(ones_f[:], 1.0)

            bq_t = cp.tile([128, 4], f32, name="bq", tag="bq")
            nc.gpsimd.dma_start(bq_t[:], bq_d[:])
            bk_t = cp.tile([128, 4], f32, name="bk", tag="bk")
            nc.gpsimd.dma_start(bk_t[:], bk_d[:])
            bvb_t = cp.tile([128, DC], f32, name="bvb", tag="bvb")
            bob_t = cp.tile([128, YC], f32, name="bob", tag="bob")

            # ---- weight + x loads ----
            # One 512KB DMA per x chunk (fewer issues = no DMA-ring
            # stalls); sync interleaves wk[c]/x[c] so the first kT group
            # starts as soon as chunk 0 lands. Nothing is issued on the
            # scalar queue (DMA issues there would block the first exp).
            # x chunks as two half-tiles [128, 1024] on sync (the first
            # halves unblock seg-0/1 projections after ~2MB); all weights
            # stream on gpsimd in parallel.
            xth = [[None, None] for _ in range(NCH)]
            for half in range(2):
                for c in range(NCH):
                    t = xp.tile([128, 1024], bf16, name=f"xt{c}_{half}",
                                tag=f"xt{c}_{half}")
                    nc.sync.dma_start(
                        t[:], xT_d[c * 128:(c + 1) * 128,
                                   half * 1024:(half + 1) * 1024])
                    xth[c][half] = t

            def xseg(c, seg):
                return xth[c][seg // 2][:, (seg % 2) * 512:
                                        (seg % 2) * 512 + 512]

            def xst(c, st):
                return xth[c][st // 8][:, (st % 8) * 128:(st % 8) * 128 + 128]

            wkch, wqch, wvch = [], [], []
            for lst, nm, w_d in ((wkch, "k", wk_d), (wqch, "q", wq_d),
                                 (wvch, "v", wv_d)):
                for c in range(NCH):
                    lst.append(pp.tile([128, DC], bf16, name=f"w{nm}{c}",
                                       tag=f"w{nm}{c}"))
            for lst, w_d in ((wkch, wk_d), (wqch, wq_d)):
                for c in range(NCH):
                    nc.gpsimd.dma_start(lst[c][:],
                                        w_d[c * 128:(c + 1) * 128, :])
            nc.gpsimd.dma_start(bvb_t[:], bvb_d[:])
            nc.gpsimd.dma_start(bob_t[:], bob_d[:])
            for c in range(NCH):
                nc.gpsimd.dma_start(wvch[c][:], wv_d[c * 128:(c + 1) * 128, :])
            woch = []
            for g in range(NCH):
                wt = pp.tile([128, YC], bf16, name=f"wo{g}", tag=f"wo{g}")
                nc.gpsimd.dma_start(wt[:], wo_d[g * 128:(g + 1) * 128, :])
                woch.append(wt)
            onesr = cp.tile([1, 128], bf16, name="onesr", tag="onesr")
            nc.vector.memset(onesr[:], 1.0)
            bobr = cp.tile([1, YC], bf16, name="bobr", tag="bobr")
            nc.vector.tensor_copy(bobr[:], bob_t[0:1, :])

            # ---- persistent projection outputs ----
            # kT/qT tile t holds local heads 2t (partitions 0:64) and
            # 2t+1 (64:128), full sequence.
            kT = [pp.tile([128, S], bf16, name=f"kT{t}", tag=f"kT{t}")
                  for t in range(4)]
            qT = [pp.tile([128, S], bf16, name=f"qT{t}", tag=f"qT{t}")
                  for t in range(4)]
            # v in augmented layout [128, HC, 65]: columns h*65..h*65+63 are
            # relu(x@wv+bv) for local head h; column h*65+64 is 1.0 (gives the
            # softmax row sums for free in the AV matmul).
            vav = [pp.tile([128, HC, 65], bf16, name=f"va{st}", tag=f"va{st}")
                   for st in range(S // 128)]

            def kq_group(wch, bias_t, out, t, seg):
                ps = psp.tile([128, 1024], f32, name="ps", tag="sc", bufs=2)
                for c in range(NCH):
                    nc.tensor.matmul(
                        ps[:, 0:512],
                        wch[c][:, t * 128:(t + 1) * 128],
                        xseg(c, seg),
                        start=(c == 0), stop=(c == NCH - 1),
                    )
                nc.vector.tensor_scalar(
                    out[:, seg * 512:(seg + 1) * 512], ps[:, 0:512],
                    bias_t[:, t:t + 1], 0.0,
                    mybir.AluOpType.add, mybir.AluOpType.max,
                )

            def v_group(st):
                ps = psp.tile([128, 1024], f32, name="ps", tag="sc", bufs=2)
                for c in range(NCH):
                    nc.tensor.matmul(
                        ps[:, 0:512],
                        xst(c, st),
                        wvch[c][:],
                        start=(c == 0), stop=(c == NCH - 1),
                    )
                nc.vector.tensor_add(ps[:, 0:512], ps[:, 0:512], bvb_t[:])
                nc.vector.tensor_scalar(
                    vav[st][:, :, 0:64],
                    ps[:, 0:512].rearrange("p (h d) -> p h d", h=HC),
                    0.0, None, mybir.AluOpType.max,
                )
                nc.vector.tensor_copy(
                    vav[st][:, :, 64:65],
                    ones_f[:, 0:8].rearrange("p (h o) -> p h o", o=1))

            # ---- AllGather plumbing for the output projection ----
            # One small pair-AllGather per (j, p). Triggers for chunk j<3
            # fire at the start of block (j+1, p): the serialized collective
            # chain (each trigger waits for the previous collective, and the
            # first waits for the ~80us cross-core startup barrier) then
            # never blocks the gpsimd queue ahead of normalize broadcasts.
            # Gathered chunk p gives global dc chunks p (even core) and 4+p
            # (odd core) — rank-uniform, no partition-id needed.
            ag_in = [dp.tile([DC, 512], bf16, name=f"agi{j}", tag=f"agi{j}")
                     for j in range(NQT)]
            ag_out = [[dp.tile([256, 512], bf16, name=f"ago{j}_{p}",
                               tag=f"ago{j}_{p}")
                       for p in range(4)] for j in range(NQT)]
            agt = {}
            RG = [[0, 1], [2, 3], [4, 5], [6, 7]]

            def ag_chunk(j, p):
                nc.gpsimd.collective_compute(
                    "AllGather", bass.mybir.AluOpType.bypass,
                    replica_groups=RG,
                    ins=[ag_in[j][p * 128:(p + 1) * 128, :].opt()],
                    outs=[ag_out[j][p][:].opt()],
                )
                # loads on sync: they block that queue ~one collective
                # (~5us), which only delays the NEXT chunk's cx write — far
                # ahead of its own collective trigger
                for (g, lo) in ((p, 0), (4 + p, 128)):
                    t = agp.tile([128, 512], bf16, name=f"agt{j}_{g}",
                                 tag=f"agt{g}", bufs=3)
                    nc.sync.dma_start(t[:], ag_out[j][p][lo:lo + 128, :])
                    agt[(j, g)] = t

            def attention(j, p, fillers):
                nblk = 4 * j + 4
                ctxA = cxp.tile([65, 512], f32, name="ctx", tag="ctx", bufs=2)
                ctxB = cxp.tile([65, 512], f32, name="ctx", tag="ctx", bufs=2)
                for kb in range(nblk):
                    dlt = kb * 128 - j * 512
                    qoff = max(dlt, 0)
                    w = 512 - qoff
                    qlo = j * 512 + qoff
                    sc = psp.tile([128, 1024], f32, name="sc",
                                  tag="sc", bufs=2)
                    e = ep.tile([128, 1024], bf16, name="e", tag="e", bufs=8)
                    for (hh, tpos) in ((0, (0, 0)), (1, (64, 0))):
                        plo = hh * 64
                        nc.tensor.matmul(
                            sc[:, hh * 512:hh * 512 + w],
                            kT[p][plo:plo + 64, kb * 128:(kb + 1) * 128],
                            qT[p][plo:plo + 64, qlo:qlo + w],
                            start=True, stop=True, tile_position=tpos,
                        )
                    nc.scalar.activation(
                        e[:].rearrange("p (h q) -> p h q", h=2)[:, :, 0:w],
                        sc[:].rearrange("p (h q) -> p h q", h=2)[:, :, 0:w],
                        AF.Exp, bias=0.0, scale=0.125,
                    )
                    if dlt >= 0:
                        nc.vector.tensor_mul(e[:, 0:128], e[:, 0:128], tri[:])
                        nc.vector.tensor_mul(
                            e[:, 512:640], e[:, 512:640], tri[:])
                    if fillers:
                        fillers.pop(0)()
                    for (hh, ctx) in ((0, ctxA), (1, ctxB)):
                        nc.tensor.matmul(
                            ctx[:, qoff:qoff + w],
                            vav[kb][:, 2 * p + hh, :],
                            e[:, hh * 512:hh * 512 + w],
                            start=(kb == 0), stop=(kb == nblk - 1),
                            skip_group_check=True,
                        )
                # normalize: evacuate psum fast with one copy, then divide by
                # the row sums (partition 64) and emit bf16 ctx for the
                # AllGather.
                # normalize. The partition-broadcast of 1/rowsum runs as a
                # rank-1 f32r matmul on the PE (the gpsimd queue blocks on
                # collective ordering and must not carry normalize work).
                # normalize. Evacuate each ctx psum bank with ONE [65,512]
                # copy (frees the bank for the next p-block's AV asap); the
                # reciprocal runs on a partition-0 copy of the sums row (a
                # base-partition-64 input breaks reciprocal_approx_fast).
                # The very last block skips the evacuation copy — nothing
                # follows it, and psum-direct shortens the tail chain.
                last = (j == 3 and p == 3)
                cx = vp.tile([128, 512], bf16, name="cx", tag="cx", bufs=3)
                for (hh, ctx) in ((0, ctxA), (1, ctxB)):
                    if last:
                        num = ctx
                    else:
                        num = sp.tile([65, 512], f32, name="cu", tag="cu",
                                      bufs=4)
                        nc.vector.tensor_copy(num[:], ctx[:])
                    rho = sp.tile([1, 512], f32, name="rho", tag="rho",
                                  bufs=4)
                    nc.vector.tensor_copy(rho[:], num[64:65, :])
                    rc1 = sp.tile([1, 512], f32, name="rc1", tag="rc1",
                                  bufs=4)
                    nc.vector.reciprocal_approx_fast(rc1[:], rho[:])
                    rcp = sp.tile([64, 512], f32, name="rcp", tag="rcp",
                                  bufs=4)
                    nc.gpsimd.partition_broadcast(rcp[:], rc1[:])
                    nc.vector.tensor_mul(
                        cx[hh * 64:hh * 64 + 64, :], num[0:64, :], rcp[:])
                nc.sync.dma_start(ag_in[j][p * 128:(p + 1) * 128, :], cx[:])

            def o_proj_group(jj, qt, tail=False):
                # contract the p=3 chunks (g=3,7) last: their AllGather is
                # the most recent and may still be in flight
                op = opp.tile([128, YC], f32, name="op", tag="op", bufs=2)
                gorder = (0, 1, 2, 4, 5, 6, 3, 7)
                for (i, g) in enumerate(gorder):
                    nc.tensor.matmul(
                        op[:],
                        agt[(jj, g)][:, qt * 128:(qt + 1) * 128],
                        woch[g][:],
                        start=(i == 0), stop=False,
                    )
                # bias via rank-1 matmul; frees the vector engine of the add
                nc.tensor.matmul(op[:], onesr[:], bobr[:],
                                 start=False, stop=True)
                ys = vp.tile([128, YC], f32, name="ys", tag="ys", bufs=2)
                if tail:
                    # scalar is idle after the last exp; vector is not
                    nc.scalar.activation(ys[:], op[:], AF.Relu,
                                         bias=0.0, scale=1.0)
                else:
                    nc.vector.tensor_scalar(
                        ys[:], op[:], 0.0, None, mybir.AluOpType.max)
                nc.sync.dma_start(
                    y_d[jj * 512 + qt * 128:jj * 512 + (qt + 1) * 128, :],
                    ys[:])

            # ---- main pipeline ----
            def K(t, seg):
                return lambda: kq_group(wkch, bk_t, kT[t], t, seg)

            def Q(t, seg):
                return lambda: kq_group(wqch, bq_t, qT[t], t, seg)

            def V(st):
                return lambda: v_group(st)

            def both(a, b):
                def f():
                    a()
                    b()
                return f

            # attention(j,p) needs kT[p]/qT[p] segs 0..j and vav 0..4j+3;
            # scores(j,p) only touch tile p, so attention(0,0) can start
            # after just K(0,0)+Q(0,0). V(kb) fillers are emitted before the
            # AV matmul of the same kb.
            K(0, 0)()
            Q(0, 0)()
            fill_plan = {
                (0, 0): [both(V(0), K(1, 0)), both(V(1), Q(1, 0)),
                         both(V(2), K(2, 0)), both(V(3), Q(2, 0))],
                (0, 1): [K(3, 0), Q(3, 0), V(4), V(5)],
                (0, 2): [K(0, 1), K(1, 1), Q(0, 1), Q(1, 1)],
                (0, 3): [K(2, 1), K(3, 1), Q(2, 1), Q(3, 1)],
                (1, 0): [V(6), V(7), K(0, 2)],
                (1, 1): [K(1, 2), K(2, 2), K(3, 2)],
                (1, 2): [Q(0, 2), Q(1, 2), Q(2, 2)],
                (1, 3): [Q(3, 2), V(8), V(9)],
                (2, 0): [V(10), V(11), K(0, 3)],
                (2, 1): [K(1, 3), K(2, 3), K(3, 3)],
                (2, 2): [Q(0, 3), Q(1, 3), Q(2, 3)],
                (2, 3): [Q(3, 3), V(12), V(13)],
                (3, 0): [V(14), V(15)],
            }

            # Collective triggers for chunk j fire at the START of block
            # (j+1, p): the serialized collective chain (each trigger waits
            # for the previous collective AND the first one waits for the
            # cross-core startup barrier, ~80us) then never blocks the
            # gpsimd queue ahead of normalize broadcasts. j=3's fire
            # immediately (the chain is warm and spaced by then).
            # o-proj placement: chunks 0-2 are deferred into the LATER,
            # scalar-bound attention phases (j=2 gets chunk 0, j=3 gets
            # chunks 1+2) where the PE has idle slots under the exp stream.
            for j in range(NQT):
                for p in range(4):
                    if j > 0:
                        ag_chunk(j - 1, p)
                    attention(j, p, fill_plan.get((j, p), []))
                    if j == 3:
                        ag_chunk(3, p)
                        o_proj_group(1, p)
                    elif j == 2:
                        o_proj_group(0, p)
            # tail: chunk 2's groups have data ready (its last gather landed
            # at block (3,3) start) and hide the final AllGather's latency
            for qt in range(4):
                o_proj_group(2, qt)
            for qt in range(4):
                o_proj_group(3, qt, tail=True)
            if debug_dump:
                nc.scalar.dma_start(dagi_d[:], ag_in[0][:])
                nc.scalar.dma_start(dago_d[:], ag_out[0][:])

    lowp.__exit__(None, None, None)
    nc.compile()
    return nc


def _get_nc():
    if "nc" not in _cache:
        _cache["nc"] = _build()
    return _cache["nc"]


def kernel(x, Wq, bq, Wk, bk, Wv, bv, Wo, bo, trace=False):
    from concourse.bass_utils import run_bass_kernel_spmd

    x = np.asarray(x, np.float32)
    Wq, bq = np.asarray(Wq, np.float32), np.asarray(bq, np.float32)
    Wk, bk = np.asarray(Wk, np.float32), np.asarray(bk, np.float32)
    Wv, bv = np.asarray(Wv, np.float32), np.asarray(bv, np.float32)
    Wo, bo = np.asarray(Wo, np.float32), np.asarray(bo, np.float32)

    nc = _get_nc()
    in_maps = []
    for c in range(NCORES):
        b, hh = c // 2, c % 2
        sl = slice(hh * DC, (hh + 1) * DC)
        yc = slice(hh * YC, (hh + 1) * YC)
        in_maps.append({
            "xT": np.ascontiguousarray(x[b].T).astype(ml_dtypes.bfloat16),
            "wq": np.ascontiguousarray(Wq[:, sl]).astype(ml_dtypes.bfloat16),
            "wk": np.ascontiguousarray(Wk[:, sl]).astype(ml_dtypes.bfloat16),
            "wv": np.ascontiguousarray(Wv[:, sl]).astype(ml_dtypes.bfloat16),
            "wo": np.ascontiguousarray(Wo[:, yc]).astype(ml_dtypes.bfloat16),
            "bq": np.ascontiguousarray(bq[sl].reshape(4, 128).T),
            "bk": np.ascontiguousarray(bk[sl].reshape(4, 128).T),
            "bvb": np.ascontiguousarray(
                np.broadcast_to(bv[sl], (128, DC))),
            "bob": np.ascontiguousarray(
                np.broadcast_to(bo[yc], (128, YC))),
        })

    res = run_bass_kernel_spmd(nc, in_maps, core_ids=list(range(NCORES)),
                               trace=trace)
    _cache["last_result"] = res

    y = np.empty((B, S, D), np.float32)
    for c in range(NCORES):
        b, hh = c // 2, c % 2
        y[b, :, hh * YC:(hh + 1) * YC] = res.results[c]["y"]
    return y


# revision 44
# speedup vs baseline: 1.0597x; 1.0124x over previous
"""Multi-head attention Trainium2 kernel (8 NeuronCores).

Sharding: data-parallel over batch (4 pairs of cores) x tensor-parallel over
heads (2-way split within each pair). Core c handles batch c//2 and heads
(c%2)*8 .. (c%2)*8+8.

Output projection: instead of row-parallel Wo + ReduceScatter (which leaves a
long serialized tail), each core AllGathers the normalized per-head context
(bf16) within its pair and computes a column shard of y = relu(ctx @ Wo + bo)
with the full D contraction — no post-matmul reduction, and the collective for
q-chunk j overlaps attention on chunk j+1. Core (b, hh) emits y[:, hh*512:
(hh+1)*512] for batch b; the host only concatenates.

Schedule: attention runs over ascending q-chunks j (512 rows each); the
projections for chunk j+1 (k/q/v segment j+1) are interleaved into the
attention kb-loop of chunk j as PE filler work, so the scalar engine (exp)
starts ~20us into the kernel instead of after all projections.

Math notes vs. the reference:
 - reference subtracts the row max (over ALL keys, pre-mask) inside exp and
   adds EPS=1e-7 to the softmax denominator. Since scores = q.k/8 >= 0
   (q,k are post-relu) and bounded (~<6), exp never overflows without the
   max subtraction, every row's denominator is >= 1, and both the max
   subtraction and EPS cancel to < 1e-5 relative. So we compute
   a = exp(s/8)*causal / sum(exp(s/8)*causal) directly.
"""

import numpy as np
import ml_dtypes

B, S, D, H = 4, 2048, 1024, 16
HD = 64          # head dim
HC = 8           # heads per core
DC = HC * HD     # 512 head-dims per core
YC = 512         # output columns per core
NCORES = 8

_cache = {}


def _build(debug_dump=False):
    import concourse.bass as bass
    import concourse.mybir as mybir
    import concourse.tile as tile
    from concourse import bacc
    from concourse.masks import make_upper_triangular

    f32 = mybir.dt.float32
    bf16 = mybir.dt.bfloat16
    AF = mybir.ActivationFunctionType

    nc = bacc.Bacc("TRN2", target_bir_lowering=False, debug=False,
                   num_devices=NCORES)

    xT_d = nc.dram_tensor("xT", [D, S], bf16, kind="ExternalInput")
    wq_d = nc.dram_tensor("wq", [D, DC], bf16, kind="ExternalInput")
    wk_d = nc.dram_tensor("wk", [D, DC], bf16, kind="ExternalInput")
    wv_d = nc.dram_tensor("wv", [D, DC], bf16, kind="ExternalInput")
    wo_d = nc.dram_tensor("wo", [D, YC], bf16, kind="ExternalInput")
    bq_d = nc.dram_tensor("bq", [128, 4], f32, kind="ExternalInput")
    bk_d = nc.dram_tensor("bk", [128, 4], f32, kind="ExternalInput")
    bvb_d = nc.dram_tensor("bvb", [128, DC], f32, kind="ExternalInput")
    bob_d = nc.dram_tensor("bob", [128, YC], f32, kind="ExternalInput")
    y_d = nc.dram_tensor("y", [S, YC], f32, kind="ExternalOutput")
    if debug_dump:
        dagi_d = nc.dram_tensor("dagi", [DC, 512], bf16,
                                kind="ExternalOutput")
        dago_d = nc.dram_tensor("dago", [2 * DC, 512], bf16,
                                kind="ExternalOutput")

    NQT = S // 512          # 4 q-chunks of 512
    NCH = D // 128          # 8 contraction chunks for projections

    lowp = nc.allow_low_precision("bf16 matmul inputs")
    lowp.__enter__()
    with tile.TileContext(nc) as tc:
        with (
            tc.tile_pool(name="const", bufs=1) as cp,
            tc.tile_pool(name="xt", bufs=1) as xp,
            tc.tile_pool(name="proj", bufs=1) as pp,
            tc.tile_pool(name="agt", bufs=2) as agp,
            tc.tile_pool(name="ework", bufs=4) as ep,
            tc.tile_pool(name="small", bufs=2) as sp,
            tc.tile_pool(name="evac", bufs=3) as vp,
            tc.tile_pool(name="ps", bufs=2, space="PSUM") as psp,
            tc.tile_pool(name="ctxps", bufs=2, space="PSUM") as cxp,
            tc.tile_pool(name="opps", bufs=2, space="PSUM") as opp,
            tc.tile_pool(name="dram", bufs=1, space="DRAM") as dp,
        ):
            # ---- constants ----
            tri = cp.tile([128, 128], bf16, name="tri", tag="tri")
            make_upper_triangular(nc, tri[:], val=1.0, diag=True)
            ones_f = cp.tile([128, 64], bf16, name="ones_f", tag="ones_f")
            nc.vector.memset(ones_f[:], 1.0)

            bq_t = cp.tile([128, 4], f32, name="bq", tag="bq")
            nc.gpsimd.dma_start(bq_t[:], bq_d[:])
            bk_t = cp.tile([128, 4], f32, name="bk", tag="bk")
            nc.gpsimd.dma_start(bk_t[:], bk_d[:])
            bvb_t = cp.tile([128, DC], f32, name="bvb", tag="bvb")
            bob_t = cp.tile([128, YC], f32, name="bob", tag="bob")

            # ---- weight + x loads ----
            # One 512KB DMA per x chunk (fewer issues = no DMA-ring
            # stalls); sync interleaves wk[c]/x[c] so the first kT group
            # starts as soon as chunk 0 lands. Nothing is issued on the
            # scalar queue (DMA issues there would block the first exp).
            # x chunks as two half-tiles [128, 1024] on sync (the first
            # halves unblock seg-0/1 projections after ~2MB); all weights
            # stream on gpsimd in parallel.
            xth = [[None, None] for _ in range(NCH)]
            for half in range(2):
                for c in range(NCH):
                    t = xp.tile([128, 1024], bf16, name=f"xt{c}_{half}",
                                tag=f"xt{c}_{half}")
                    nc.sync.dma_start(
                        t[:], xT_d[c * 128:(c + 1) * 128,
                                   half * 1024:(half + 1) * 1024])
                    xth[c][half] = t

            def xseg(c, seg):
                return xth[c][seg // 2][:, (seg % 2) * 512:
                                        (seg % 2) * 512 + 512]

            def xst(c, st):
                return xth[c][st // 8][:, (st % 8) * 128:(st % 8) * 128 + 128]

            wkch, wqch, wvch = [], [], []
            for lst, nm, w_d in ((wkch, "k", wk_d), (wqch, "q", wq_d),
                                 (wvch, "v", wv_d)):
                for c in range(NCH):
                    lst.append(pp.tile([128, DC], bf16, name=f"w{nm}{c}",
                                       tag=f"w{nm}{c}"))
            for lst, w_d in ((wkch, wk_d), (wqch, wq_d)):
                for c in range(NCH):
                    nc.gpsimd.dma_start(lst[c][:],
                                        w_d[c * 128:(c + 1) * 128, :])
            nc.gpsimd.dma_start(bvb_t[:], bvb_d[:])
            nc.gpsimd.dma_start(bob_t[:], bob_d[:])
            for c in range(NCH):
                nc.gpsimd.dma_start(wvch[c][:], wv_d[c * 128:(c + 1) * 128, :])
            woch = []
            for g in range(NCH):
                wt = pp.tile([128, YC], bf16, name=f"wo{g}", tag=f"wo{g}")
                nc.gpsimd.dma_start(wt[:], wo_d[g * 128:(g + 1) * 128, :])
                woch.append(wt)
            # emitted late (at j=1) — a copy waiting on the bob DMA here
            # would block the vector queue ahead of the first relu work
            onesr = cp.tile([1, 128], bf16, name="onesr", tag="onesr")
            bobr = cp.tile([1, YC], bf16, name="bobr", tag="bobr")

            # ---- persistent projection outputs ----
            # kT/qT tile t holds local heads 2t (partitions 0:64) and
            # 2t+1 (64:128), full sequence.
            kT = [pp.tile([128, S], bf16, name=f"kT{t}", tag=f"kT{t}")
                  for t in range(4)]
            qT = [pp.tile([128, S], bf16, name=f"qT{t}", tag=f"qT{t}")
                  for t in range(4)]
            # v in augmented layout [128, HC, 65]: columns h*65..h*65+63 are
            # relu(x@wv+bv) for local head h; column h*65+64 is 1.0 (gives the
            # softmax row sums for free in the AV matmul).
            vav = [pp.tile([128, HC, 65], bf16, name=f"va{st}", tag=f"va{st}")
                   for st in range(S // 128)]

            def kq_group(wch, bias_t, out, t, seg):
                ps = psp.tile([128, 1024], f32, name="ps", tag="sc", bufs=2)
                for c in range(NCH):
                    nc.tensor.matmul(
                        ps[:, 0:512],
                        wch[c][:, t * 128:(t + 1) * 128],
                        xseg(c, seg),
                        start=(c == 0), stop=(c == NCH - 1),
                    )
                nc.vector.tensor_scalar(
                    out[:, seg * 512:(seg + 1) * 512], ps[:, 0:512],
                    bias_t[:, t:t + 1], 0.0,
                    mybir.AluOpType.add, mybir.AluOpType.max,
                )

            def v_group(st):
                ps = psp.tile([128, 1024], f32, name="ps", tag="sc", bufs=2)
                for c in range(NCH):
                    nc.tensor.matmul(
                        ps[:, 0:512],
                        xst(c, st),
                        wvch[c][:],
                        start=(c == 0), stop=(c == NCH - 1),
                    )
                nc.vector.tensor_add(ps[:, 0:512], ps[:, 0:512], bvb_t[:])
                nc.vector.tensor_scalar(
                    vav[st][:, :, 0:64],
                    ps[:, 0:512].rearrange("p (h d) -> p h d", h=HC),
                    0.0, None, mybir.AluOpType.max,
                )
                nc.vector.tensor_copy(
                    vav[st][:, :, 64:65],
                    ones_f[:, 0:8].rearrange("p (h o) -> p h o", o=1))

            # ---- AllGather plumbing for the output projection ----
            # One small pair-AllGather per (j, p). Triggers for chunk j<3
            # fire at the start of block (j+1, p): the serialized collective
            # chain (each trigger waits for the previous collective, and the
            # first waits for the ~80us cross-core startup barrier) then
            # never blocks the gpsimd queue ahead of normalize broadcasts.
            # Gathered chunk p gives global dc chunks p (even core) and 4+p
            # (odd core) — rank-uniform, no partition-id needed.
            ag_in = [dp.tile([DC, 512], bf16, name=f"agi{j}", tag=f"agi{j}")
                     for j in range(NQT)]
            ag_out = [[dp.tile([256, 512], bf16, name=f"ago{j}_{p}",
                               tag=f"ago{j}_{p}")
                       for p in range(4)] for j in range(NQT)]
            agt = {}
            RG = [[0, 1], [2, 3], [4, 5], [6, 7]]

            ag_full2 = dp.tile([2 * DC, 512], bf16, name="agf2", tag="agf2")

            def ag_chunk_full2():
                # chunk 2 as ONE gather (its input is complete by block
                # (3,0)): keeps the j=3-region collective chain short so the
                # tail's per-p gathers don't queue behind it
                nc.gpsimd.collective_compute(
                    "AllGather", bass.mybir.AluOpType.bypass,
                    replica_groups=RG,
                    ins=[ag_in[2][:].opt()],
                    outs=[ag_full2[:].opt()],
                )
                for g in range(NCH):
                    t = agp.tile([128, 512], bf16, name=f"agt2_{g}",
                                 tag=f"agt{g}", bufs=3)
                    nc.sync.dma_start(
                        t[:], ag_full2[g * 128:(g + 1) * 128, :])
                    agt[(2, g)] = t

            def ag_chunk(j, p):
                nc.gpsimd.collective_compute(
                    "AllGather", bass.mybir.AluOpType.bypass,
                    replica_groups=RG,
                    ins=[ag_in[j][p * 128:(p + 1) * 128, :].opt()],
                    outs=[ag_out[j][p][:].opt()],
                )
                # loads on sync: they block that queue ~one collective
                # (~5us), which only delays the NEXT chunk's cx write — far
                # ahead of its own collective trigger
                for (g, lo) in ((p, 0), (4 + p, 128)):
                    t = agp.tile([128, 512], bf16, name=f"agt{j}_{g}",
                                 tag=f"agt{g}", bufs=3)
                    nc.sync.dma_start(t[:], ag_out[j][p][lo:lo + 128, :])
                    agt[(j, g)] = t

            def attention(j, p, fillers):
                nblk = 4 * j + 4
                ctxA = cxp.tile([65, 512], f32, name="ctx", tag="ctx", bufs=2)
                ctxB = cxp.tile([65, 512], f32, name="ctx", tag="ctx", bufs=2)
                for kb in range(nblk):
                    dlt = kb * 128 - j * 512
                    qoff = max(dlt, 0)
                    w = 512 - qoff
                    qlo = j * 512 + qoff
                    sc = psp.tile([128, 1024], f32, name="sc",
                                  tag="sc", bufs=2)
                    e = ep.tile([128, 1024], bf16, name="e", tag="e", bufs=8)
                    for (hh, tpos) in ((0, (0, 0)), (1, (64, 0))):
                        plo = hh * 64
                        nc.tensor.matmul(
                            sc[:, hh * 512:hh * 512 + w],
                            kT[p][plo:plo + 64, kb * 128:(kb + 1) * 128],
                            qT[p][plo:plo + 64, qlo:qlo + w],
                            start=True, stop=True, tile_position=tpos,
                        )
                    nc.scalar.activation(
                        e[:].rearrange("p (h q) -> p h q", h=2)[:, :, 0:w],
                        sc[:].rearrange("p (h q) -> p h q", h=2)[:, :, 0:w],
                        AF.Exp, bias=0.0, scale=0.125,
                    )
                    if dlt >= 0:
                        nc.vector.tensor_mul(e[:, 0:128], e[:, 0:128], tri[:])
                        nc.vector.tensor_mul(
                            e[:, 512:640], e[:, 512:640], tri[:])
                    if fillers:
                        fillers.pop(0)()
                    for (hh, ctx) in ((0, ctxA), (1, ctxB)):
                        nc.tensor.matmul(
                            ctx[:, qoff:qoff + w],
                            vav[kb][:, 2 * p + hh, :],
                            e[:, hh * 512:hh * 512 + w],
                            start=(kb == 0), stop=(kb == nblk - 1),
                            skip_group_check=True,
                        )
                # normalize: evacuate psum fast with one copy, then divide by
                # the row sums (partition 64) and emit bf16 ctx for the
                # AllGather.
                # normalize. The partition-broadcast of 1/rowsum runs as a
                # rank-1 f32r matmul on the PE (the gpsimd queue blocks on
                # collective ordering and must not carry normalize work).
                # normalize. Evacuate each ctx psum bank with ONE [65,512]
                # copy (frees the bank for the next p-block's AV asap); the
                # reciprocal runs on a partition-0 copy of the sums row (a
                # base-partition-64 input breaks reciprocal_approx_fast).
                # The very last block skips the evacuation copy — nothing
                # follows it, and psum-direct shortens the tail chain.
                last = (j == 3 and p == 3)
                cx = vp.tile([128, 512], bf16, name="cx", tag="cx", bufs=3)
                for (hh, ctx) in ((0, ctxA), (1, ctxB)):
                    if last:
                        num = ctx
                    else:
                        num = sp.tile([65, 512], f32, name="cu", tag="cu",
                                      bufs=4)
                        nc.vector.tensor_copy(num[:], ctx[:])
                    rho = sp.tile([1, 512], f32, name="rho", tag="rho",
                                  bufs=4)
                    nc.vector.tensor_copy(rho[:], num[64:65, :])
                    rc1 = sp.tile([1, 512], f32, name="rc1", tag="rc1",
                                  bufs=4)
                    nc.vector.reciprocal_approx_fast(rc1[:], rho[:])
                    rcp = sp.tile([64, 512], f32, name="rcp", tag="rcp",
                                  bufs=4)
                    nc.gpsimd.partition_broadcast(rcp[:], rc1[:])
                    nc.vector.tensor_mul(
                        cx[hh * 64:hh * 64 + 64, :], num[0:64, :], rcp[:])
                nc.sync.dma_start(ag_in[j][p * 128:(p + 1) * 128, :], cx[:])

            def o_proj_group(jj, qt, tail=False):
                # contract the p=3 chunks (g=3,7) last: their AllGather is
                # the most recent and may still be in flight
                op = opp.tile([128, YC], f32, name="op", tag="op", bufs=2)
                gorder = (0, 1, 2, 4, 5, 6, 3, 7)
                for (i, g) in enumerate(gorder):
                    nc.tensor.matmul(
                        op[:],
                        agt[(jj, g)][:, qt * 128:(qt + 1) * 128],
                        woch[g][:],
                        start=(i == 0), stop=False,
                    )
                # bias via rank-1 matmul; frees the vector engine of the add
                nc.tensor.matmul(op[:], onesr[:], bobr[:],
                                 start=False, stop=True)
                ys = vp.tile([128, YC], f32, name="ys", tag="ys", bufs=2)
                if tail:
                    # scalar is idle after the last exp; vector is not
                    nc.scalar.activation(ys[:], op[:], AF.Relu,
                                         bias=0.0, scale=1.0)
                else:
                    nc.vector.tensor_scalar(
                        ys[:], op[:], 0.0, None, mybir.AluOpType.max)
                nc.sync.dma_start(
                    y_d[jj * 512 + qt * 128:jj * 512 + (qt + 1) * 128, :],
                    ys[:])

            # ---- main pipeline ----
            def K(t, seg):
                return lambda: kq_group(wkch, bk_t, kT[t], t, seg)

            def Q(t, seg):
                return lambda: kq_group(wqch, bq_t, qT[t], t, seg)

            def V(st):
                return lambda: v_group(st)

            def both(a, b):
                def f():
                    a()
                    b()
                return f

            # attention(j,p) needs kT[p]/qT[p] segs 0..j and vav 0..4j+3;
            # scores(j,p) only touch tile p, so attention(0,0) can start
            # after just K(0,0)+Q(0,0). V(kb) fillers are emitted before the
            # AV matmul of the same kb.
            K(0, 0)()
            Q(0, 0)()
            fill_plan = {
                (0, 0): [both(V(0), K(1, 0)), both(V(1), Q(1, 0)),
                         both(V(2), K(2, 0)), both(V(3), Q(2, 0))],
                (0, 1): [K(3, 0), Q(3, 0), V(4), V(5)],
                (0, 2): [K(0, 1), K(1, 1), Q(0, 1), Q(1, 1)],
                (0, 3): [K(2, 1), K(3, 1), Q(2, 1), Q(3, 1)],
                (1, 0): [V(6), V(7), K(0, 2)],
                (1, 1): [K(1, 2), K(2, 2), K(3, 2)],
                (1, 2): [Q(0, 2), Q(1, 2), Q(2, 2)],
                (1, 3): [Q(3, 2), V(8), V(9)],
                (2, 0): [V(10), V(11), K(0, 3)],
                (2, 1): [K(1, 3), K(2, 3), K(3, 3)],
                (2, 2): [Q(0, 3), Q(1, 3), Q(2, 3)],
                (2, 3): [Q(3, 3), V(12), V(13)],
                (3, 0): [V(14), V(15)],
            }

            # Collective triggers for chunk j fire at the START of block
            # (j+1, p): the serialized collective chain (each trigger waits
            # for the previous collective AND the first one waits for the
            # cross-core startup barrier, ~80us) then never blocks the
            # gpsimd queue ahead of normalize broadcasts. j=3's fire
            # immediately (the chain is warm and spaced by then).
            # o-proj placement: chunks 0-2 are deferred into the LATER,
            # scalar-bound attention phases (j=2 gets chunk 0, j=3 gets
            # chunks 1+2) where the PE has idle slots under the exp stream.
            for j in range(NQT):
                for p in range(4):
                    if j == 1 and p == 0:
                        nc.vector.memset(onesr[:], 1.0)
                        nc.vector.tensor_copy(bobr[:], bob_t[0:1, :])
                    if j == 3:
                        if p == 0:
                            ag_chunk_full2()
                    elif j > 0:
                        ag_chunk(j - 1, p)
                    attention(j, p, fill_plan.get((j, p), []))
                    if j == 3:
                        ag_chunk(3, p)
                        o_proj_group(1, p)
                    elif j == 2:
                        o_proj_group(0, p)
            # tail: chunk 2's groups have data ready (its last gather landed
            # at block (3,3) start) and hide the final AllGather's latency
            for qt in range(4):
                o_proj_group(2, qt)
            for qt in range(4):
                o_proj_group(3, qt, tail=True)
            if debug_dump:
                nc.scalar.dma_start(dagi_d[:], ag_in[0][:])
                nc.scalar.dma_start(dago_d[:], ag_out[0][:])

    lowp.__exit__(None, None, None)
    nc.compile()
    return nc


def _get_nc():
    if "nc" not in _cache:
        _cache["nc"] = _build()
    return _cache["nc"]


def kernel(x, Wq, bq, Wk, bk, Wv, bv, Wo, bo, trace=False):
    from concourse.bass_utils import run_bass_kernel_spmd

    x = np.asarray(x, np.float32)
    Wq, bq = np.asarray(Wq, np.float32), np.asarray(bq, np.float32)
    Wk, bk = np.asarray(Wk, np.float32), np.asarray(bk, np.float32)
    Wv, bv = np.asarray(Wv, np.float32), np.asarray(bv, np.float32)
    Wo, bo = np.asarray(Wo, np.float32), np.asarray(bo, np.float32)

    nc = _get_nc()
    in_maps = []
    for c in range(NCORES):
        b, hh = c // 2, c % 2
        sl = slice(hh * DC, (hh + 1) * DC)
        yc = slice(hh * YC, (hh + 1) * YC)
        in_maps.append({
            "xT": np.ascontiguousarray(x[b].T).astype(ml_dtypes.bfloat16),
            "wq": np.ascontiguousarray(Wq[:, sl]).astype(ml_dtypes.bfloat16),
            "wk": np.ascontiguousarray(Wk[:, sl]).astype(ml_dtypes.bfloat16),
            "wv": np.ascontiguousarray(Wv[:, sl]).astype(ml_dtypes.bfloat16),
            "wo": np.ascontiguousarray(Wo[:, yc]).astype(ml_dtypes.bfloat16),
            "bq": np.ascontiguousarray(bq[sl].reshape(4, 128).T),
            "bk": np.ascontiguousarray(bk[sl].reshape(4, 128).T),
            "bvb": np.ascontiguousarray(
                np.broadcast_to(bv[sl], (128, DC))),
            "bob": np.ascontiguousarray(
                np.broadcast_to(bo[yc], (128, YC))),
        })

    res = run_bass_kernel_spmd(nc, in_maps, core_ids=list(range(NCORES)),
                               trace=trace)
    _cache["last_result"] = res

    y = np.empty((B, S, D), np.float32)
    for c in range(NCORES):
        b, hh = c // 2, c % 2
        y[b, :, hh * YC:(hh + 1) * YC] = res.results[c]["y"]
    return y
